# revision 10
# baseline (speedup 1.0000x reference)
"""Trainium2 Bass kernel: 5x5 grayscale dilation (flat all-ones SE) =
5x5 stride-1 max-pool with geodesic border, over [8,3,2048,2048] f32.

Strategy (pure data-parallel over batch, 1 image per NeuronCore; the
active path is build_eo_nc, v3; build_tall_nc is the v2 fallback):
- Inputs are non-negative, so the -1e4 geodesic pad is replaced by
  zero-padding (exact) and the host quantizes x*255 to uint8 (max
  commutes with monotone quantization; err <= 1/510 << 2e-2 tol).
  uint8 loads halve input DMA; the idle Act engine upconverts to bf16
  on device (0..255 integers are exact in bf16). Compute runs in bf16:
  DVE 2x_1p mode (2-byte dtype + unit-stride innermost AP) gives
  2 elem/cycle. Output is stored bf16 and upcast exactly on host.
- "Tall image" layout: 3 channels stacked with 4 zero separator rows
  -> [6156, W]. 128 partitions x 48-row bands cover all 6144 output
  rows with fully uniform compute; the 2 bands straddling a channel
  boundary load 56 rows (others 52) and store in two pieces.
- Shared-pair window-5 max in BOTH directions (~2.06 ops/elem each
  instead of 3): rows via step-2 middle-AP-dim slicing (innermost stays
  unit-stride, 2x_1p holds); columns via host-deinterleaved even/odd
  column planes, re-interleaved on host after.
- DRAM pre-swizzled on host into per-W-tile (256 cols) plane arrays so
  each partition's row-block is ONE contiguous multi-KB DMA descriptor:
  HWDGE descriptor generation (~9ns/desc) and per-packet overhead are
  off the critical path entirely.
- Engine duty split: DVE maxes only; Act converts u8->bf16 (pipelined
  one tile ahead, plane E first); sync triggers loads (HWDGE); GPSIMD
  triggers stores (SWDGE) so they never block the Act queue.
- Steady state is DVE-bound at ~99% occupancy: ~237.6us/rep measured
  (vs 641us baseline). cv is double-buffered so the convs run a full
  tile ahead (single-buffered cv squeezed them into the post-read
  window and cost ~1.2us/tile).
"""

import sys

import numpy as np

for _p in ("/opt/trn_rl_repo",):
    if _p not in sys.path:
        sys.path.insert(0, _p)

NEG = -10000.0  # reference MAX_VAL border (host fallback only)

# tall-image geometry (C=3, H=2048, W=2048 hardcoded)
C, H, W = 3, 2048, 2048
SEP = 4          # zero rows between channels (>= window-1)
PADT = 2         # zero rows top/bottom, zero cols left/right
TALL = C * H + (C - 1) * SEP + 2 * PADT   # 6156
WP = W + 2 * PADT                          # 2052
HSUB = 48        # output rows per partition band
NPART = 128      # HSUB * NPART == C*H + straddle slack
HH = 56          # loaded rows for straddle bands
HLOAD = 52       # loaded rows for normal bands (48 + 4 halo)
ROUT = 52        # rows produced by the uniform H-pass

# v4 straddle-free geometry: 42 whole bands per channel in the main
# pass (rows 0..2015), the last 32 rows of each channel go to a
# column-banded "strip" pass.
NB = 42            # main-pass bands per channel
NMAIN = 3 * NB     # 126 used partitions in the main pass
SROWS = H - NB * HSUB   # 32 strip output rows per channel
SLOAD = SROWS + 4       # 36 loaded strip rows
SCOL = W // NPART       # 16 strip output cols per partition
SHO = SCOL // 2         # 8  strip plane output cols
SHF = SHO + 2           # 10 strip plane cols with halo


def _band_tables(C=C, H=H, npart=NPART):
    """Load/store DMA groups for the tall-image banding.

    Band p covers output rows [48p, 48p+48) of the flat [C*H] output
    (channel-major). Source tall-row start s = 48p + SEP*c(48p), affine
    within a channel. Bands that straddle a channel boundary load HH
    rows (others HLOAD) and store in two pieces.

    Returns (lgroups, sgroups):
      lgroups: (part0, nparts, src_row0, nrows)   [contiguous, affine]
      sgroups: (part0, nparts, tile_row0, nrows, out_row0)
    """
    lgroups, sgroups = [], []
    run_start, run_chan = None, None

    def flush(p_end):
        nonlocal run_start
        if run_start is not None:
            q = run_start
            lgroups.append((q, p_end - q, HSUB * q + SEP * run_chan, HLOAD))
            sgroups.append((q, p_end - q, 0, HSUB, HSUB * q))
            run_start = None

    for p in range(npart):
        o0 = HSUB * p
        c0, c1 = o0 // H, (o0 + HSUB - 1) // H
        if c0 == c1:
            if run_start is not None and c0 != run_chan:
                flush(p)  # channel boundary aligned with band boundary
            if run_start is None:
                run_start, run_chan = p, c0
        else:
            flush(p)
            lgroups.append((p, 1, o0 + SEP * c0, HH))
            n0 = c1 * H - o0
            sgroups.append((p, 1, 0, n0, o0))
            sgroups.append((p, 1, n0 + SEP, HSUB - n0, c1 * H))
    flush(npart)
    return lgroups, sgroups


def build_tall_nc(wt=256, reps=1, geo=None):
    """Single-core Bass program: [n_wt, TALL, wt+4] bf16 (pre-swizzled
    W-tiles with halo) -> [n_wt, C*H, wt] bf16.

    The host pre-splits the padded tall image into W-tiles so that each
    partition's whole row-block is one contiguous DRAM run -> one big
    (~27 KB) DMA descriptor per partition instead of one per row. This
    keeps HWDGE descriptor generation (~9 ns/desc) and per-packet DMA
    overhead off the critical path entirely.
    """
    from contextlib import ExitStack

    import concourse.mybir as mybir
    import concourse.tile as tile
    from concourse import bacc
    from bass_rust import AP

    Cg, Hg, Wg = geo if geo else (C, H, W)
    npart = Cg * Hg // HSUB
    assert npart * HSUB == Cg * Hg and npart <= 128
    tall = Cg * Hg + (Cg - 1) * SEP + 2 * PADT

    bf16 = mybir.dt.bfloat16
    n_wt = Wg // wt
    assert n_wt * wt == Wg
    Wt = wt + 4

    nc = bacc.Bacc()
    img = nc.declare_dram_parameter("image", [n_wt, tall, Wt], bf16,
                                    isOutput=False)
    outp = nc.declare_dram_parameter("out", [n_wt, Cg * Hg, wt], bf16,
                                     isOutput=True)

    lgroups, sgroups = _band_tables(Cg, Hg, npart)

    ppitch = HH * Wt       # in-tile per-partition elements
    opitch = ROUT * wt     # out-tile per-partition elements

    with tile.TileContext(nc) as tc, ExitStack() as ctx:
        pin = ctx.enter_context(tc.tile_pool(name="pin", bufs=2))
        pp = ctx.enter_context(tc.tile_pool(name="pp", bufs=1))
        pt1 = ctx.enter_context(tc.tile_pool(name="pt1", bufs=1))
        pR = ctx.enter_context(tc.tile_pool(name="pR", bufs=1))
        # out is written only by the final W-pass op (tile end), so a
        # single buffer gives the store a full tile-time to drain.
        pout = ctx.enter_context(tc.tile_pool(name="pout", bufs=1))

        for _rep in range(reps):
            for wi in range(n_wt):
                in_t = pin.tile([npart, HH, Wt], bf16)
                base = in_t[:]
                # zero the never-loaded halo rows (52:56) of normal bands
                # so the uniform H-pass reads defined data. Engine ops must
                # start at partition 0 (mod 32), so zero the full range and
                # let the straddle loads overwrite (Tile serializes the
                # WAW). Per tile: cross-iteration reads of a bufs=1 tile
                # aren't tracked, so a once-only memzero is racy.
                nc.scalar.memzero(in_t[:, HLOAD:HH, :])
                for gi, (p0, np_, srow, nrows) in enumerate(lgroups):
                    # one contiguous (nrows*Wt)-elem run per partition
                    sap = [[HSUB * Wt, np_], [1, nrows * Wt]]
                    dap = [[ppitch, np_], [1, nrows * Wt]]
                    src = AP(img, (wi * tall + srow) * Wt, sap)
                    dst = AP(base.tensor, base.offset + p0 * ppitch, dap)
                    nc.sync.dma_start(out=dst, in_=src)

                out_t = pout.tile([npart, ROUT, wt], bf16)
                p = pp.tile([npart, HH // 2, Wt], bf16)
                t1 = pt1.tile([npart, HH // 2 - 1, Wt], bf16)
                R = pR.tile([npart, ROUT, Wt], bf16)
                # H-pass (rows): shared-pair window-5 max
                nc.vector.tensor_max(p[:], in_t[:, 0:HH:2, :],
                                     in_t[:, 1:HH:2, :])
                nc.vector.tensor_max(t1[:], p[:, 0:27, :], p[:, 1:28, :])
                nc.vector.tensor_max(R[:, 0:ROUT:2, :], t1[:, 0:26, :],
                                     in_t[:, 4:HH - 1:2, :])
                nc.vector.tensor_max(R[:, 1:ROUT:2, :], t1[:, 1:27, :],
                                     in_t[:, 1:HH - 3:2, :])
                # W-pass (cols): cascade 2,3,5
                u = pp.tile([npart, ROUT, Wt - 1], bf16)
                nc.vector.tensor_max(u[:], R[:, :, 0:Wt - 1], R[:, :, 1:Wt])
                v = pR.tile([npart, ROUT, Wt - 2], bf16)
                nc.vector.tensor_max(v[:], u[:, :, 0:Wt - 2],
                                     u[:, :, 1:Wt - 1])
                nc.vector.tensor_max(out_t[:], v[:, :, 0:wt],
                                     v[:, :, 2:wt + 2])

                ob = out_t[:]
                for gi, (p0, np_, r0, nrows, orow) in enumerate(sgroups):
                    src = AP(ob.tensor, ob.offset + p0 * opitch + r0 * wt,
                             [[opitch, np_], [1, nrows * wt]])
                    dst = AP(outp, (wi * Cg * Hg + orow) * wt,
                             [[HSUB * wt, np_], [1, nrows * wt]])
                    nc.scalar.dma_start(out=dst, in_=src)
    return nc


def build_eo_nc(wt=256, reps=1, geo=None):
    """v3: even/odd column planes + uint8 loads.

    DRAM in:  [n_wt, 2, tall, wt/2+2] uint8 (host-quantized x*255,
              plane 0 = even padded cols, plane 1 = odd).
    DRAM out: [n_wt, 2, C*H, wt/2] bf16 (planes re-interleaved on host).

    The deinterleave makes the shared-pair trick work in the W direction
    too (all unit-stride): pw=max(E,O), t1w=max(pw,pw<<1),
    outE=max(t1w, E<<2), outO=max(t1w<<1, O) => ~2 ops/elem instead
    of 3. uint8 loads halve input DMA bytes; the Act engine upconverts
    to bf16 (0..255 integers are exact in bf16) while DVE works on the
    previous tile.
    """
    from contextlib import ExitStack

    import concourse.mybir as mybir
    import concourse.tile as tile
    from concourse import bacc
    from bass_rust import AP

    Cg, Hg, Wg = geo if geo else (C, H, W)
    npart = Cg * Hg // HSUB
    assert npart * HSUB == Cg * Hg and npart <= 128
    tall = Cg * Hg + (Cg - 1) * SEP + 2 * PADT

    bf16 = mybir.dt.bfloat16
    u8 = mybir.dt.uint8
    n_wt = Wg // wt
    assert n_wt * wt == Wg and wt % 2 == 0
    hf = wt // 2 + 2          # plane cols (with 1-pair halo each side)
    ho = wt // 2              # plane output cols

    hfp = (hf + 3) & ~3       # u8 plane width padded to 4B multiple
                              # (memzero's uint32 bitcast needs it)

    nc = bacc.Bacc()
    img = nc.declare_dram_parameter("image", [n_wt, 2, tall, hfp], u8,
                                    isOutput=False)
    outp = nc.declare_dram_parameter("out", [n_wt, 2, Cg * Hg, ho], bf16,
                                     isOutput=True)

    lgroups, sgroups = _band_tables(Cg, Hg, npart)

    ippitch = 2 * HH * hfp    # u8 in-tile per-partition elements
    cpitch = HH * hfp         # per-plane pitch inside the u8 in-tile
    opitch = 2 * ROUT * ho    # out-tile per-partition elements

    with tile.TileContext(nc) as tc, ExitStack() as ctx:
        pin = ctx.enter_context(tc.tile_pool(name="pin", bufs=2))
        # cv double-buffered: the u8->bf16 convs for tile i+1 then run a
        # full tile ahead instead of squeezing into the window after the
        # last cv read of tile i (which cost ~1.2us of DVE stall per tile)
        pcv = ctx.enter_context(tc.tile_pool(name="pcv", bufs=2))
        pp = ctx.enter_context(tc.tile_pool(name="pp", bufs=1))
        pt1 = ctx.enter_context(tc.tile_pool(name="pt1", bufs=1))
        pR = ctx.enter_context(tc.tile_pool(name="pR", bufs=1))
        pout = ctx.enter_context(tc.tile_pool(name="pout", bufs=1))

        for _rep in range(reps):
            for wi in range(n_wt):
                in_t = pin.tile([npart, 2, HH, hfp], u8)
                base = in_t[:]
                # zero rows 52:56 (never loaded for normal bands) so the
                # uniform H-pass reads defined data for them.
                nc.scalar.memzero(in_t[:, :, HLOAD:HH, :])
                for pl in range(2):
                    for p0, np_, srow, nrows in lgroups:
                        sap = [[HSUB * hfp, np_], [1, nrows * hfp]]
                        dap = [[ippitch, np_], [1, nrows * hfp]]
                        src = AP(img, ((wi * 2 + pl) * tall + srow) * hfp,
                                 sap)
                        dst = AP(base.tensor,
                                 base.offset + p0 * ippitch + pl * cpitch,
                                 dap)
                        nc.sync.dma_start(out=dst, in_=src)

                cv = pcv.tile([npart, 2, HH, hf], bf16)
                R = pR.tile([npart, 2, ROUT, hf], bf16)
                p = pp.tile([npart, 28, hf], bf16)
                t1 = pt1.tile([npart, 27, hf], bf16)
                # per plane: convert u8->bf16 then H-pass (rows); plane E
                # first so conv(E, i+1) can start while plane O of tile i
                # is still in the H-pass (Act/DVE pipelining).
                for pl in range(2):
                    nc.scalar.copy(cv[:, pl, :, :], in_t[:, pl, :, 0:hf])
                    nc.vector.tensor_max(p[:], cv[:, pl, 0:HH:2, :],
                                         cv[:, pl, 1:HH:2, :])
                    nc.vector.tensor_max(t1[:], p[:, 0:27, :], p[:, 1:28, :])
                    nc.vector.tensor_max(R[:, pl, 0:ROUT:2, :],
                                         t1[:, 0:26, :],
                                         cv[:, pl, 4:HH - 1:2, :])
                    nc.vector.tensor_max(R[:, pl, 1:ROUT:2, :],
                                         t1[:, 1:27, :],
                                         cv[:, pl, 1:HH - 3:2, :])
                # W-pass (cols), shared-pair across planes; each plane's
                # stores are emitted right after its final op so the
                # drain starts before the other plane finishes.
                out_t = pout.tile([npart, 2, ROUT, ho], bf16)
                pw = pp.tile([npart, ROUT, hf], bf16)
                t1w = pt1.tile([npart, ROUT, hf - 1], bf16)
                nc.vector.tensor_max(pw[:], R[:, 0, :, :], R[:, 1, :, :])
                nc.vector.tensor_max(t1w[:], pw[:, :, 0:hf - 1],
                                     pw[:, :, 1:hf])
                ob = out_t[:]

                def emit_stores(pl):
                    for p0, np_, r0, nrows, orow in sgroups:
                        src = AP(ob.tensor,
                                 ob.offset + p0 * opitch
                                 + pl * ROUT * ho + r0 * ho,
                                 [[opitch, np_], [1, nrows * ho]])
                        dst = AP(outp, ((wi * 2 + pl) * Cg * Hg + orow) * ho,
                                 [[HSUB * ho, np_], [1, nrows * ho]])
                        # stores on GPSIMD SWDGE: keeps the Act queue free
                        # for convs (conv(i+1) must not sit behind
                        # stores(i), which are gated on tile-i's end)
                        nc.gpsimd.dma_start(out=dst, in_=src)

                nc.vector.tensor_max(out_t[:, 0, :, :], t1w[:, :, 0:ho],
                                     R[:, 0, :, 2:2 + ho])
                emit_stores(0)
                nc.vector.tensor_max(out_t[:, 1, :, :], t1w[:, :, 1:1 + ho],
                                     R[:, 1, :, 0:ho])
                emit_stores(1)
    return nc


def build_eo_nc4(wt=256, reps=1):
    """v4: straddle-free banding + remainder strip.

    Main pass: 42 bands x 48 rows per channel (126 partitions), each
    band entirely inside one channel -> uniform 52-row loads, 48-row
    H-output, 48-row W-pass (v3 processed 56/52 rows to accommodate
    the 2 channel-straddling bands: ~7.6% DVE slack).  The last 32
    rows of each channel are handled by a single column-banded strip
    pass (128 partitions x 16 cols, ~2% of pixels).  H-pass ops are
    merged across the two column planes (4D APs) to halve instruction
    count.

    DRAM in : image [n_wt, 2, tall, hfp] u8 (as v3), strip [128, 2160] u8
    DRAM out: out [n_wt, 2, C*H, ho] bf16 (rows 2016.. of each channel
              unwritten), strip_out [128, 1536] bf16
    """
    from contextlib import ExitStack

    import concourse.mybir as mybir
    import concourse.tile as tile
    from concourse import bacc
    from bass_rust import AP

    bf16 = mybir.dt.bfloat16
    u8 = mybir.dt.uint8
    n_wt = W // wt
    assert n_wt * wt == W and wt % 2 == 0
    hf = wt // 2 + 2
    ho = wt // 2
    hfp = (hf + 3) & ~3
    tall = TALL
    HL = HLOAD              # 52 loaded rows per main band
    RO = HSUB               # 48 H-output rows

    s_in_len = 2 * 3 * SLOAD * SHF     # 2160 u8 per partition
    s_out_len = 2 * 3 * SROWS * SHO    # 1536 bf16 per partition

    nc = bacc.Bacc()
    img = nc.declare_dram_parameter("image", [n_wt, 2, tall, hfp], u8,
                                    isOutput=False)
    sdram = nc.declare_dram_parameter("strip", [NPART, s_in_len], u8,
                                      isOutput=False)
    outp = nc.declare_dram_parameter("out", [n_wt, 2, C * H, ho], bf16,
                                     isOutput=True)
    sout = nc.declare_dram_parameter("strip_out", [NPART, s_out_len], bf16,
                                     isOutput=True)

    ippitch = 2 * HL * hfp   # u8 in-tile per-partition elements
    cpitch = HL * hfp        # per-plane pitch inside the u8 in-tile
    opitch = 2 * RO * ho     # out-tile per-partition elements

    with tile.TileContext(nc) as tc, ExitStack() as ctx:
        pin = ctx.enter_context(tc.tile_pool(name="pin", bufs=3))
        pcv = ctx.enter_context(tc.tile_pool(name="pcv", bufs=2))
        pp = ctx.enter_context(tc.tile_pool(name="pp", bufs=1))
        pt1 = ctx.enter_context(tc.tile_pool(name="pt1", bufs=1))
        pR = ctx.enter_context(tc.tile_pool(name="pR", bufs=1))
        pout = ctx.enter_context(tc.tile_pool(name="pout", bufs=1))
        ps = ctx.enter_context(tc.tile_pool(name="ps", bufs=1))

        for _rep in range(reps):
            for wi in range(n_wt):
                in_t = pin.tile([NPART, 2, HL, hfp], u8)
                base = in_t[:]
                for c in range(C):
                    # both planes in one 3D-AP DMA (plane is a middle dim)
                    srow = c * (H + SEP)   # = PADT + c*(H+SEP) - 2
                    sap = [[HSUB * hfp, NB], [tall * hfp, 2],
                           [1, HL * hfp]]
                    dap = [[ippitch, NB], [cpitch, 2], [1, HL * hfp]]
                    src = AP(img, (wi * 2 * tall + srow) * hfp, sap)
                    dst = AP(base.tensor,
                             base.offset + c * NB * ippitch, dap)
                    nc.sync.dma_start(out=dst, in_=src)

                # u8 -> bf16 for both planes in one ACTIVATE
                cv = pcv.tile([NPART, 2, HL, hf], bf16)
                nc.scalar.copy(cv[:], in_t[:, :, :, 0:hf])

                # H-pass, plane-merged 4D ops
                p = pp.tile([NPART, 2, HL // 2, hf], bf16)
                t1 = pt1.tile([NPART, 2, HL // 2 - 1, hf], bf16)
                R = pR.tile([NPART, 2, RO, hf], bf16)
                nc.vector.tensor_max(p[:], cv[:, :, 0:HL:2, :],
                                     cv[:, :, 1:HL:2, :])
                nc.vector.tensor_max(t1[:], p[:, :, 0:25, :],
                                     p[:, :, 1:26, :])
                nc.vector.tensor_max(R[:, :, 0:RO:2, :], t1[:, :, 0:24, :],
                                     cv[:, :, 4:HL:2, :])
                nc.vector.tensor_max(R[:, :, 1:RO:2, :], t1[:, :, 1:25, :],
                                     cv[:, :, 1:RO + 1:2, :])

                # W-pass
                out_t = pout.tile([NPART, 2, RO, ho], bf16)
                pw = pp.tile([NPART, RO, hf], bf16)
                t1w = pt1.tile([NPART, RO, hf - 1], bf16)
                nc.vector.tensor_max(pw[:], R[:, 0, :, :], R[:, 1, :, :])
                nc.vector.tensor_max(t1w[:], pw[:, :, 0:hf - 1],
                                     pw[:, :, 1:hf])
                ob = out_t[:]

                def emit_stores():
                    # both planes in one 3D-AP DMA per channel group
                    for c in range(C):
                        src = AP(ob.tensor,
                                 ob.offset + c * NB * opitch,
                                 [[opitch, NB], [RO * ho, 2],
                                  [1, RO * ho]])
                        dst = AP(outp, (wi * 2 * C * H + c * H) * ho,
                                 [[HSUB * ho, NB], [C * H * ho, 2],
                                  [1, RO * ho]])
                        nc.gpsimd.dma_start(out=dst, in_=src)

                nc.vector.tensor_max(out_t[:, 0, :, :], t1w[:, :, 0:ho],
                                     R[:, 0, :, 2:2 + ho])
                nc.vector.tensor_max(out_t[:, 1, :, :], t1w[:, :, 1:1 + ho],
                                     R[:, 1, :, 0:ho])
                emit_stores()

            # ---- remainder strip: last 32 rows of each channel ----
            # SBUF is tight: alias dead-by-then strip tiles via shared
            # pool tags (in_s dead after convs -> R_s reuses its slot;
            # cv_s dead after R assembly -> out_s reuses; p/t1 as main).
            in_s = ps.tile([NPART, 2, 3, SLOAD, SHF], u8, tag="sE")
            bs = in_s[:]
            nc.sync.dma_start(
                out=AP(bs.tensor, bs.offset, [[s_in_len, NPART],
                                              [1, s_in_len]]),
                in_=AP(sdram, 0, [[s_in_len, NPART], [1, s_in_len]]))
            cv_s = ps.tile([NPART, 2, 3, SLOAD, SHF], bf16, tag="sB")
            p_s = ps.tile([NPART, 2, 3, SLOAD // 2, SHF], bf16, tag="sC")
            t1_s = ps.tile([NPART, 2, 3, SLOAD // 2 - 1, SHF], bf16,
                           tag="sD")
            R_s = ps.tile([NPART, 2, 3, SROWS, SHF], bf16, tag="sE")
            for pl in range(2):
                # both convs before any R_s write (R_s aliases in_s)
                nc.scalar.copy(cv_s[:, pl], in_s[:, pl])
            for pl in range(2):
                nc.vector.tensor_max(p_s[:, pl], cv_s[:, pl, :, 0:SLOAD:2, :],
                                     cv_s[:, pl, :, 1:SLOAD:2, :])
                nc.vector.tensor_max(t1_s[:, pl], p_s[:, pl, :, 0:17, :],
                                     p_s[:, pl, :, 1:18, :])
                nc.vector.tensor_max(R_s[:, pl, :, 0:SROWS:2, :],
                                     t1_s[:, pl, :, 0:16, :],
                                     cv_s[:, pl, :, 4:SLOAD:2, :])
                nc.vector.tensor_max(R_s[:, pl, :, 1:SROWS:2, :],
                                     t1_s[:, pl, :, 1:17, :],
                                     cv_s[:, pl, :, 1:SROWS + 1:2, :])
            pw_s = ps.tile([NPART, 3, SROWS, SHF], bf16, tag="sC")
            t1w_s = ps.tile([NPART, 3, SROWS, SHF - 1], bf16, tag="sD")
            out_s = ps.tile([NPART, 2, 3, SROWS, SHO], bf16, tag="sB")
            nc.vector.tensor_max(pw_s[:], R_s[:, 0], R_s[:, 1])
            nc.vector.tensor_max(t1w_s[:], pw_s[:, :, :, 0:SHF - 1],
                                 pw_s[:, :, :, 1:SHF])
            nc.vector.tensor_max(out_s[:, 0], t1w_s[:, :, :, 0:SHO],
                                 R_s[:, 0, :, :, 2:2 + SHO])
            nc.vector.tensor_max(out_s[:, 1], t1w_s[:, :, :, 1:1 + SHO],
                                 R_s[:, 1, :, :, 0:SHO])
            obs = out_s[:]
            nc.gpsimd.dma_start(
                out=AP(sout, 0, [[s_out_len, NPART], [1, s_out_len]]),
                in_=AP(obs.tensor, obs.offset, [[s_out_len, NPART],
                                                [1, s_out_len]]))
    return nc


def _padded_u8(image):
    """[C,H,W] f32 in [0,1] -> zero-padded tall u8 buffer [TALL, WP]."""
    buf = np.zeros((TALL, WP), dtype=np.uint8)
    q = np.rint(image * 255.0).astype(np.uint8)
    for c in range(C):
        r0 = PADT + c * (H + SEP)
        buf[r0:r0 + H, PADT:PADT + W] = q[c]
    return buf


def pack_strip(buf):
    """Padded tall u8 buf -> strip input [128, 2160] u8.

    strip[p, pl, c, r, j] = buf[PADT + c*(H+SEP) + (H-SROWS) - 2 + r,
                                16p + 2j + pl]
    """
    from numpy.lib.stride_tricks import sliding_window_view

    out = np.zeros((NPART, 2, 3, SLOAD, SHF), np.uint8)
    for c in range(C):
        r0 = PADT + c * (H + SEP) + (H - SROWS) - 2
        sub = buf[r0:r0 + SLOAD, :]
        for pl in range(2):
            spl = sub[:, pl::2]                       # [SLOAD, 1026]
            win = sliding_window_view(spl, SHF, axis=1)
            sel = win[:, 0:8 * NPART:8, :]            # [SLOAD, 128, SHF]
            out[:, pl, c] = sel.transpose(1, 0, 2)
    return out.reshape(NPART, -1)


def unpack_strip(strip_bf16, full):
    """Scatter strip_out [128, 1536] bf16 into full [C, H, W] f32."""
    u = np.ascontiguousarray(strip_bf16).view(np.uint16)
    f = (u.astype(np.uint32) << 16).view(np.float32).reshape(
        NPART, 2, 3, SROWS, SHO)
    for c in range(C):
        for pl in range(2):
            plane = f[:, pl, c].transpose(1, 0, 2).reshape(SROWS, W // 2)
            full[c, H - SROWS:H, pl::2] = plane * (1.0 / 255.0)


def _tile_eo(buf, wt=256):
    """Padded tall u8 buf [TALL, WP] -> [n_wt, 2, TALL, hfp] u8 planes."""
    n_wt = W // wt
    hf = wt // 2 + 2
    hfp = (hf + 3) & ~3
    til = np.zeros((n_wt, 2, TALL, hfp), dtype=np.uint8)
    for ti in range(n_wt):
        x = buf[:, ti * wt:ti * wt + wt + 4]
        til[ti, 0, :, :hf] = x[:, 0::2]
        til[ti, 1, :, :hf] = x[:, 1::2]
    return til


def pack_eo(image, geo=None, wt=256):
    """[C,H,W] f32 in [0,1] -> u8-quantized even/odd planes
    [n_wt, 2, tall, wt/2+2]."""
    assert geo is None or tuple(geo) == (C, H, W)
    return _tile_eo(_padded_u8(image), wt)


def unpack_eo(planes_bf16, geo=None, wt=256):
    """[n_wt, 2, C*H, wt/2] bf16 (0..255 scale) -> [C,H,W] f32."""
    Cg, Hg, Wg = geo if geo else (C, H, W)
    n_wt = Wg // wt
    u = np.ascontiguousarray(planes_bf16).view(np.uint16)
    f = (u.astype(np.uint32) << 16).view(np.float32).reshape(
        n_wt, 2, Cg * Hg, wt // 2)
    full = np.empty((Cg * Hg, Wg), dtype=np.float32)
    for ti in range(n_wt):
        full[:, ti * wt:ti * wt + wt:2] = f[ti, 0]
        full[:, ti * wt + 1:ti * wt + wt:2] = f[ti, 1]
    return (full * (1.0 / 255.0)).reshape(Cg, Hg, Wg)


def _numpy_ref(image, se):
    """Slow exact fallback for a non-all-ones structuring element."""
    B, Ci, Hi, Wi = image.shape
    kh, kw = se.shape
    oy, ox = kh // 2, kw // 2
    pad = np.full((B, Ci, Hi + kh - 1, Wi + kw - 1), NEG, dtype=image.dtype)
    pad[:, :, oy:oy + Hi, ox:ox + Wi] = image
    neigh = np.where(se == 0, NEG, 0.0).astype(image.dtype)[::-1, ::-1]
    out = np.full((B, Ci, Hi, Wi), -np.inf, dtype=image.dtype)
    for i in range(kh):
        for j in range(kw):
            np.maximum(out, pad[:, :, i:i + Hi, j:j + Wi] + neigh[i, j], out)
    return out


def pack_host(image, geo=None, wt=256):
    """[C,H,W] f32 (non-negative) -> pre-swizzled W-tiled padded bf16
    [n_wt, tall, wt+4]."""
    import ml_dtypes

    Cg, Hg, Wg = geo if geo else (C, H, W)
    tall = Cg * Hg + (Cg - 1) * SEP + 2 * PADT
    wp = Wg + 2 * PADT
    buf = np.zeros((tall, wp), dtype=ml_dtypes.bfloat16)
    bf = image.astype(ml_dtypes.bfloat16)
    for c in range(Cg):
        r0 = PADT + c * (Hg + SEP)
        buf[r0:r0 + Hg, PADT:PADT + Wg] = bf[c]
    n_wt = Wg // wt
    til = np.empty((n_wt, tall, wt + 4), dtype=ml_dtypes.bfloat16)
    for ti in range(n_wt):
        til[ti] = buf[:, ti * wt:ti * wt + wt + 4]
    return til


def unpack_host(tiled_bf16, geo=None):
    """[n_wt, C*H, wt] bf16 -> [C,H,W] f32 (exact upcast)."""
    Cg, Hg, Wg = geo if geo else (C, H, W)
    flat = np.concatenate(
        [np.ascontiguousarray(t) for t in tiled_bf16], axis=1)
    u = flat.view(np.uint16).astype(np.uint32) << 16
    return u.view(np.float32).reshape(Cg, Hg, Wg)


_CACHE = {}


def kernel(image, kernel):
    image = np.asarray(image, dtype=np.float32)
    se = np.asarray(kernel, dtype=np.float32)
    B = image.shape[0] if image.ndim == 4 else 0
    if (se.shape != (5, 5) or np.any(se == 0) or image.ndim != 4
            or image.shape[1:] != (C, H, W) or B != 8
            or image.min() < 0 or image.max() > 1.0):
        return _numpy_ref(image, se)

    from concourse.bass_utils import run_bass_kernel_spmd

    if "nc" not in _CACHE:
        nc0 = build_eo_nc4()
        if not nc0.is_finalized():
            nc0.finalize()
        _CACHE["nc"] = nc0
    nc = _CACHE["nc"]

    in_maps = []
    for i in range(B):
        buf = _padded_u8(image[i])
        in_maps.append({"image": _tile_eo(buf), "strip": pack_strip(buf)})
    res = run_bass_kernel_spmd(nc, in_maps, list(range(B)))
    outs = []
    for i in range(B):
        full = unpack_eo(np.asarray(res.results[i]["out"]))
        unpack_strip(np.asarray(res.results[i]["strip_out"]), full)
        outs.append(full)
    return np.stack(outs, axis=0)


if __name__ == "__main__":
    rng = np.random.default_rng(0)
    image = rng.random((8, 3, 2048, 2048), dtype=np.float32)
    se = np.ones((5, 5), np.float32)
    out = kernel(image, se)
    ref = _numpy_ref(image, se)
    rel = (np.abs(out - ref) / np.maximum(np.abs(ref), 1e-6)).max()
    print("rel max err:", rel)

